# revision 1
# baseline (speedup 1.0000x reference)
"""Bahdanau attention (B=32, S=2048, ENC2=1024, ATT=512) on 8 TRN2
NeuronCores, data-parallel over batch (4 batches/core), weights replicated.

Per-core program (Bass/Tile, fp32 in/out, f32r matmuls):
  Ws   = dec @ W_w + W_b + U_b              (prologue, PE)
  encT = per-[128,128] PE transposes of enc (f32r transpose mode)
  UhT  = U_w^T-chunk @ encT                 [a 128, s 512] psum tiles
  tanh = tanh(UhT + Ws) via ACT bias fusion (per-partition bias)
  en   = v^T tanh                           (PE, deferred 1 s-block)
  alpha= exp(en)/sum exp(en)                (ACT exp + fused row-sum)

Output alpha [32, 2048] fp32, gathered from the 8 cores.
"""

import numpy as np

import concourse.bass as bass
import concourse.mybir as mybir
import concourse.tile as tile
from concourse import bacc
from concourse.masks import make_identity

F32 = mybir.dt.float32
F32R = mybir.dt.float32r

N_CORES = 8
B_FULL, S, E, A = 32, 2048, 1024, 512
B_SH = B_FULL // N_CORES          # 4 batches per core
SBLK = 512                        # s-block (matmul N)
N_SBLK = S // SBLK                # 4 per batch
EJ = E // 128                     # 8 e-chunks
AM = A // 128                     # 4 a-chunks
CC = SBLK // 128                  # 4 s-subchunks per s-block


def r(ap):
    return ap.bitcast(F32R)


def build_program(reps=1, no_dma=False, no_transpose=False, no_mm=False,
                  x4_bufs=2, dma_split=False):
    nc = bacc.Bacc("TRN2", target_bir_lowering=False, debug=False,
                   num_devices=N_CORES)

    dec = nc.dram_tensor("decoder_hidden", [B_SH, E], F32R, kind="ExternalInput")
    enc = nc.dram_tensor("encoder_all_hidden", [B_SH, S, E], F32R,
                         kind="ExternalInput")
    W_w = nc.dram_tensor("W_w", [E, A], F32R, kind="ExternalInput")
    W_b = nc.dram_tensor("W_b", [A], F32R, kind="ExternalInput")
    U_w = nc.dram_tensor("U_w", [E, A], F32R, kind="ExternalInput")
    U_b = nc.dram_tensor("U_b", [A], F32R, kind="ExternalInput")
    v_w = nc.dram_tensor("v_w", [A, 1], F32R, kind="ExternalInput")
    alpha = nc.dram_tensor("alpha", [B_SH, S], F32, kind="ExternalOutput")

    with tile.TileContext(nc) as tc:
        with (
            tc.tile_pool(name="const", bufs=1) as constp,
            tc.tile_pool(name="x4", bufs=x4_bufs) as x4p,
            tc.tile_pool(name="enct", bufs=24) as enctp,
            tc.tile_pool(name="tanh", bufs=8) as tanhp,
            tc.tile_pool(name="epi", bufs=2) as epip,
            tc.tile_pool(name="psT", bufs=3, space="PSUM") as psTp,
            tc.tile_pool(name="psUh", bufs=3, space="PSUM") as psUhp,
            tc.tile_pool(name="psE", bufs=2, space="PSUM") as psEp,
        ):
            # ---------------- prologue, part 1 (tiny + first enc DMA) --
            ident_f32 = constp.tile([128, 128], F32, tag="identf")
            make_identity(nc, ident_f32)
            ident = constp.tile([128, 128], F32R, tag="ident")
            nc.vector.tensor_copy(ident, ident_f32)

            dec_sb = constp.tile([B_SH, E], F32R, tag="dec")
            nc.sync.dma_start(dec_sb, dec[:, :])
            wb_sb = constp.tile([1, A], F32R, tag="wb")
            nc.sync.dma_start(wb_sb, W_b[None, :])
            ub_sb = constp.tile([1, A], F32R, tag="ub")
            nc.sync.dma_start(ub_sb, U_b[None, :])
            v_sb = constp.tile([128, AM], F32R, tag="v")
            nc.sync.dma_start(v_sb.rearrange("p (c o) -> p c o", c=AM),
                              v_w.rearrange("(c p) o -> p c o", p=128))

            # dec transposes first: PE work that's ready immediately
            dect = constp.tile([128, B_SH * EJ], F32R, tag="dect")
            for j in range(EJ):
                pst = psTp.tile([128, 128], F32R, tag="psT")
                nc.tensor.transpose(r(pst[:, :B_SH]),
                                    r(dec_sb[:, 128 * j:128 * (j + 1)]),
                                    r(ident[:B_SH, :B_SH]))
                nc.scalar.copy(dect[:, B_SH * j:B_SH * (j + 1)], pst[:, :B_SH])

            # first enc block DMA goes out before the big weight loads,
            # split per 128-row chunk so transposes can start on chunk 0
            x4_first = x4p.tile([128, CC * E], F32R, tag="x4")
            if not no_dma:
                for c in range(CC):
                    nc.sync.dma_start(x4_first[:, E * c:E * (c + 1)],
                                      enc[0, 128 * c:128 * (c + 1), :])
            else:
                nc.vector.memset(x4_first.bitcast(F32)[:, :1], 0.0)

            # W_w before U_w (Ws matmuls unblock first), U_w split per
            # chunk so MM j-groups can start as chunks land
            ww = constp.tile([128, EJ * A], F32R, tag="ww")
            nc.sync.dma_start(ww.rearrange("e (j a) -> e j a", j=EJ),
                              W_w.rearrange("(j e) a -> e j a", e=128))
            uw = constp.tile([128, EJ * A], F32R, tag="uw")
            for j in range(EJ):
                nc.sync.dma_start(uw[:, A * j:A * (j + 1)],
                                  U_w[128 * j:128 * (j + 1), :])

            bias_sum = constp.tile([1, A], F32R, tag="bias")
            nc.vector.tensor_tensor(out=bias_sum, in0=wb_sb, in1=ub_sb,
                                    op=mybir.AluOpType.add)
            ones14f = constp.tile([1, B_SH], F32, tag="onesf")
            nc.vector.memset(ones14f, 1.0)
            ones14 = constp.tile([1, B_SH], F32R, tag="ones")
            nc.vector.tensor_copy(ones14, ones14f)

            wst = constp.tile([128, AM * B_SH], F32, tag="wst")

            enct_const = None
            if no_transpose:
                enct_const = [constp.tile([128, SBLK], F32R, tag=f"enctc{jj}",
                                          name=f"enctc_{jj}")
                              for jj in range(EJ)]
                for jj in range(EJ):
                    nc.vector.tensor_copy(enct_const[jj], uw[:, :SBLK])

            def prologue_part2():
                # Ws = dec @ W_w + (W_b + U_b):  psum [B_SH, A]
                ps_ws = psEp.tile([B_SH, A], F32, tag="psE", name="ps_ws")
                for j in range(EJ):
                    nc.tensor.matmul(ps_ws,
                                     r(dect[:, B_SH * j:B_SH * (j + 1)]),
                                     r(ww[:, A * j:A * (j + 1)]),
                                     start=(j == 0), stop=False)
                nc.tensor.matmul(ps_ws, r(ones14), r(bias_sum),
                                 start=False, stop=True)
                ws_sb = constp.tile([B_SH, A], F32R, tag="ws", name="ws_sb")
                nc.scalar.copy(ws_sb, ps_ws)
                # WsT [128 a', (m b)]: col 4m+b = Ws[b, 128m + p]
                for m in range(AM):
                    pst = psTp.tile([128, 128], F32R, tag="psT",
                                    name=f"pst_ws_{m}")
                    nc.tensor.transpose(r(pst[:, :B_SH]),
                                        r(ws_sb[:, 128 * m:128 * (m + 1)]),
                                        r(ident[:B_SH, :B_SH]))
                    nc.scalar.copy(wst[:, B_SH * m:B_SH * (m + 1)],
                                   pst[:, :B_SH])

            # ---------------- main loop ----------------
            evac_k = 0
            for rep in range(reps):
              for b in range(B_SH):
                # deferred energy-MM work: list of (s0, [tanh tiles])
                pending = []
                exp_b = epip.tile([1, S], F32, tag="exp",
                                  name=f"exp_{rep}_{b}")
                den_b = epip.tile([1, N_SBLK], F32, tag="den",
                                  name=f"den_{rep}_{b}")

                def flush_energy(pending, exp_b=exp_b, den_b=den_b, b=b,
                                 rep=rep):
                    s0, pths = pending.pop(0)
                    sb = s0 // SBLK
                    ps_e = psEp.tile([1, SBLK], F32, tag="psE",
                                     name=f"psE_{rep}_{b}_{sb}")
                    for m in range(AM):
                        nc.tensor.matmul(ps_e, r(v_sb[:, m:m + 1]),
                                         r(pths[m]),
                                         start=(m == 0), stop=(m == AM - 1))
                    # exp of this s-block + its partial denominator
                    nc.scalar.activation(out=exp_b[:, s0:s0 + SBLK],
                                         in_=ps_e,
                                         func=mybir.ActivationFunctionType.Exp,
                                         accum_out=den_b[:, sb:sb + 1])

                for sblk in range(N_SBLK):
                    s0 = SBLK * sblk
                    if b == 0 and sblk == 0 and rep == 0:
                        x4 = x4_first
                    else:
                        x4 = x4p.tile([128, CC * E], F32R, tag="x4")
                        if no_dma:
                            nc.vector.memset(x4.bitcast(F32)[:, :1], 0.0)
                        elif dma_split:
                            h = CC // 2
                            nc.sync.dma_start(
                                x4[:, :h * E].rearrange(
                                    "p (c e) -> p c e", c=h),
                                enc[b, s0:s0 + 128 * h, :]
                                .rearrange("(c p) e -> p c e", p=128))
                            nc.scalar.dma_start(
                                x4[:, h * E:].rearrange(
                                    "p (c e) -> p c e", c=CC - h),
                                enc[b, s0 + 128 * h:s0 + SBLK, :]
                                .rearrange("(c p) e -> p c e", p=128))
                        else:
                            nc.sync.dma_start(
                                x4.rearrange("p (c e) -> p c e", c=CC),
                                enc[b, s0:s0 + SBLK, :]
                                .rearrange("(c p) e -> p c e", p=128))

                    # transpose enc block: encT_j [e 128, s 512]
                    # 4 transposes land in one [128,512] psum tile (one
                    # bank), evacuated by a single wide copy.
                    if no_transpose:
                        enct = enct_const
                    else:
                        enct = [enctp.tile([128, SBLK], F32R, tag="enct",
                                           name=f"enct_{rep}_{b}_{sblk}_{jj}")
                                for jj in range(EJ)]
                    if no_transpose:
                        pass
                    elif b == 0 and sblk == 0 and rep == 0:
                        # c-major + narrow evacs: start compute as soon as
                        # the first 128-row DMA chunk lands
                        for c in range(CC):
                            for j in range(EJ):
                                pstn = psTp.tile([128, 128], F32R, tag="psT",
                                                 name=f"pstn_{rep}_{c}_{j}")
                                nc.tensor.transpose(
                                    r(pstn),
                                    r(x4[:, E * c + 128 * j:
                                         E * c + 128 * (j + 1)]),
                                    r(ident))
                                dst = enct[j][:, 128 * c:128 * (c + 1)]
                                if (c * EJ + j) % 5 == 4:
                                    nc.scalar.copy(dst, pstn)
                                else:
                                    nc.vector.tensor_copy(dst, pstn)
                    else:
                        for j in range(EJ):
                            pst = psTp.tile([128, SBLK], F32R, tag="psT")
                            for c in range(CC):
                                nc.tensor.transpose(
                                    r(pst[:, 128 * c:128 * (c + 1)]),
                                    r(x4[:, E * c + 128 * j:
                                         E * c + 128 * (j + 1)]),
                                    r(ident))
                            if evac_k % 5 == 4:
                                nc.scalar.copy(enct[j], pst)
                            else:
                                nc.vector.tensor_copy(enct[j], pst)
                            evac_k += 1

                    if b == 0 and sblk == 0 and rep == 0:
                        # Ws chain runs on PE while block-0 evacs drain
                        prologue_part2()

                    if no_mm:
                        continue
                    # UhT per a-chunk; tanh with fused +Ws bias
                    ths = []
                    for m in range(AM):
                        ps_uh = psUhp.tile([128, SBLK], F32, tag="psUh")
                        for j in range(EJ):
                            nc.tensor.matmul(
                                ps_uh,
                                r(uw[:, A * j + 128 * m:A * j + 128 * (m + 1)]),
                                r(enct[j]),
                                start=(j == 0), stop=(j == EJ - 1))
                        th = tanhp.tile([128, SBLK], F32R, tag="tanh",
                                        name=f"tanh_{rep}_{b}_{sblk}_{m}")
                        nc.scalar.activation(
                            out=th, in_=ps_uh,
                            func=mybir.ActivationFunctionType.Tanh,
                            bias=wst[:, B_SH * m + b:B_SH * m + b + 1])
                        ths.append(th)
                    pending.append((s0, ths))

                    # energy MMs + exp for the PREVIOUS s-block (tanh long
                    # done, so the PE never stalls on ACT)
                    if len(pending) > 1:
                        flush_energy(pending)
                    if b == B_SH - 1 and sblk == N_SBLK - 1:
                        # shorten the kernel tail: flush the final block now
                        flush_energy(pending)
                if no_mm:
                    al0 = epip.tile([1, S], F32, tag="al",
                                    name=f"al_{rep}_{b}")
                    nc.vector.memset(al0, 0.0)
                    nc.sync.dma_start(alpha[b:b + 1, :], al0)
                    continue
                if pending:
                    flush_energy(pending)

                # softmax epilogue (no max subtraction; |energy| <= 22.6)
                dsum_b = epip.tile([1, 1], F32, tag="dsum",
                                   name=f"dsum_{rep}_{b}")
                nc.vector.reduce_sum(dsum_b, den_b,
                                     axis=mybir.AxisListType.X)
                inv_b = epip.tile([1, 1], F32, tag="inv",
                                  name=f"inv_{rep}_{b}")
                nc.vector.reciprocal(inv_b, dsum_b)
                al_b = epip.tile([1, S], F32, tag="al",
                                 name=f"al_{rep}_{b}")
                nc.vector.tensor_scalar_mul(al_b, exp_b, inv_b)
                nc.sync.dma_start(alpha[b:b + 1, :], al_b)

    nc.compile()
    return nc


def shard_inputs(inputs):
    """Full inputs dict -> list of 8 per-core input dicts."""
    dec = np.ascontiguousarray(inputs["decoder_hidden"], dtype=np.float32)
    enc = np.ascontiguousarray(inputs["encoder_all_hidden"], dtype=np.float32)
    base = {
        "W_w": np.ascontiguousarray(inputs["W_w"], dtype=np.float32),
        "W_b": np.ascontiguousarray(inputs["W_b"], dtype=np.float32),
        "U_w": np.ascontiguousarray(inputs["U_w"], dtype=np.float32),
        "U_b": np.ascontiguousarray(inputs["U_b"], dtype=np.float32),
        "v_w": np.ascontiguousarray(inputs["v_w"], dtype=np.float32),
    }
    maps = []
    for c in range(N_CORES):
        m = dict(base)
        m["decoder_hidden"] = dec[c * B_SH:(c + 1) * B_SH]
        m["encoder_all_hidden"] = enc[c * B_SH:(c + 1) * B_SH]
        maps.append(m)
    return maps


_NC_CACHE = None


def get_program():
    global _NC_CACHE
    if _NC_CACHE is None:
        _NC_CACHE = build_program()
    return _NC_CACHE


def kernel(**inputs):
    from concourse import bass_utils
    nc = get_program()
    maps = shard_inputs(inputs)
    res = bass_utils.run_bass_kernel_spmd(nc, maps,
                                          core_ids=list(range(N_CORES)))
    return np.concatenate([res.results[c]["alpha"] for c in range(N_CORES)],
                          axis=0)

